# revision 1
# baseline (speedup 1.0000x reference)
"""Trainium2 Bass kernel for nn_EvaluatorNetwork.

Network (per sample):
  sep = per-column spectral decomposition of image  -> (128, 128, 128)
  x = concat([sep, mask_embedding]) -> (134, 128, 128)
  conv0 4x4 s2 (134->256) + b0, lrelu          -> (256, 64, 64)
  conv1 4x4 s2 (256->512), inorm, lrelu        -> (512, 32, 32)
  conv2 4x4 s2 (512->1024), inorm, lrelu       -> (1024, 16, 16)
  conv3 4x4 s2 (1024->1024), inorm, lrelu      -> (1024, 8, 8)
  avgpool -> (1024,); head 1024->128 + b4      -> (128,)

Sharding: pure data parallel, batch 8 over 8 NeuronCores; weights replicated.

Math notes:
  sep[i,h,w] = colRT[i,h]*cos(2pi*i*w/W) + colJT[i,h]*sin(2pi*i*w/W)
    colRT = C @ img^T, colJT = S @ img^T,  C/S[w',i] = cos/sin(2pi*i*w'/W)/W
  b1..b3 cancel exactly through instance norm (constant channel shift), so
  they are ignored; b0 and b4 are applied.
  lrelu(y) = 0.2*y + 0.8*relu(y); relu(0.8*y) = 0.8*relu(y) (used so the
  per-partition-scale Relu activation can produce the relu part directly).
"""
from contextlib import ExitStack

import numpy as np

import concourse.bass as bass
import concourse.tile as tile
from concourse import bacc, mybir
from concourse.masks import make_identity

F32 = mybir.dt.float32
F16 = mybir.dt.float16

B, H, W = 8, 128, 128
EPS = 1e-5

# conv output spatial sizes
S0, S1, S2, S3 = 64, 32, 16, 8


def _build_nc():
    nc = bacc.Bacc("TRN2", target_bir_lowering=False, debug=False)

    # ---------------- DRAM parameters (per-core) ----------------
    d_img = nc.dram_tensor("img", [H, W], F32, kind="ExternalInput")
    d_maskim = nc.dram_tensor("maskim", [96, S0, S0], F16, kind="ExternalInput")
    d_C = nc.dram_tensor("twC", [W, W], F32, kind="ExternalInput")
    d_S = nc.dram_tensor("twS", [W, W], F32, kind="ExternalInput")
    d_c2 = nc.dram_tensor("twc2", [W, W], F16, kind="ExternalInput")
    d_s2 = nc.dram_tensor("tws2", [W, W], F16, kind="ExternalInput")
    d_w0s = nc.dram_tensor("w0s", [2, 16, 128, 128], F16, kind="ExternalInput")
    d_wm = nc.dram_tensor("wm", [96, 256], F16, kind="ExternalInput")
    d_w1 = nc.dram_tensor("w1l", [4, 2, 128, 16, 128], F16, kind="ExternalInput")
    d_w2 = nc.dram_tensor("w2l", [8, 4, 128, 16, 128], F16, kind="ExternalInput")
    d_w3 = nc.dram_tensor("w3l", [8, 8, 128, 16, 128], F16, kind="ExternalInput")
    d_w4 = nc.dram_tensor("w4l", [8, 128, 128], F16, kind="ExternalInput")
    d_b0 = nc.dram_tensor("b0t", [128, 4], F32, kind="ExternalInput")  # [b0, 0.8*b0]
    d_b4 = nc.dram_tensor("b4t", [128, 1], F32, kind="ExternalInput")
    d_out = nc.dram_tensor("out", [128], F32, kind="ExternalOutput")

    from contextlib import contextmanager

    @contextmanager
    def low_priority(tc, bump):
        orig = tc.cur_priority
        tc.cur_priority = orig + bump
        try:
            yield
        finally:
            tc.cur_priority = orig

    with tile.TileContext(nc) as tc, ExitStack() as ctx:
        const = ctx.enter_context(tc.tile_pool(name="const", bufs=1))
        act = ctx.enter_context(tc.tile_pool(name="act", bufs=1))
        wch = ctx.enter_context(tc.tile_pool(name="wch", bufs=21))
        ps = ctx.enter_context(tc.tile_pool(name="ps", bufs=3, space="PSUM"))
        tmp = ctx.enter_context(tc.tile_pool(name="tmp", bufs=6))
        tsp = ctx.enter_context(tc.tile_pool(name="tsp", bufs=3))

        # ---------------- constants / inputs ----------------
        ident = const.tile([128, 128], F32)
        make_identity(nc, ident[:])
        img32 = const.tile([128, 128], F32)
        nc.sync.dma_start(img32[:], d_img.ap())
        twC = const.tile([128, 128], F32)
        nc.sync.dma_start(twC[:], d_C.ap())
        twS = const.tile([128, 128], F32)
        nc.sync.dma_start(twS[:], d_S.ap())
        c2ated = const.tile([128, 128], F16)
        nc.sync.dma_start(c2ated[:], d_c2.ap())
        s2ated = const.tile([128, 128], F16)
        nc.sync.dma_start(s2ated[:], d_s2.ap())
        b0t = const.tile([128, 4], F32)
        nc.sync.dma_start(b0t[:], d_b0.ap())
        b4t = const.tile([128, 1], F32)
        nc.sync.dma_start(b4t[:], d_b4.ap())

        # mask im2col (host-computed): rows (kh,kw,ci), cols (oh,ow)
        mask_im = act.tile([96, S0, S0], F16)
        nc.sync.dma_start(mask_im[:], d_maskim.ap())

        # small weights resident
        w0s_sb = const.tile([128, 2, 16, 128], F16)
        src = d_w0s.ap().rearrange("m t k c -> k (m t) c")
        nc.sync.dma_start(w0s_sb[:].rearrange("k m t c -> k (m t) c"), src)
        wm_sb = const.tile([96, 256], F16)
        nc.sync.dma_start(wm_sb[:], d_wm.ap())
        w4_sb = const.tile([128, 8, 128], F16)
        nc.sync.dma_start(w4_sb[:], d_w4.ap().rearrange("t k c -> k t c"))

        # ---------------- spectral map ----------------
        pT = ps.tile([128, 128], F32, tag="ps")
        nc.tensor.transpose(pT[:], img32[:], ident[:])
        imgT32 = const.tile([128, 128], F32)
        nc.vector.tensor_copy(imgT32[:], pT[:])

        pR = ps.tile([128, 128], F32, tag="ps")
        nc.tensor.matmul(pR[:], twC[:], imgT32[:], start=True, stop=True)
        colRT = const.tile([128, 128], F16)
        nc.vector.tensor_copy(colRT[:], pR[:])
        pJ = ps.tile([128, 128], F32, tag="ps")
        nc.tensor.matmul(pJ[:], twS[:], imgT32[:], start=True, stop=True)
        colJT = const.tile([128, 128], F16)
        nc.vector.tensor_copy(colJT[:], pJ[:])

        sep_pad = act.tile([128, H + 2, W + 2], F16)
        # zero only the border strips (interior fully written below)
        nc.gpsimd.memset(sep_pad[:, 0, :], 0.0)
        nc.gpsimd.memset(sep_pad[:, H + 1, :], 0.0)
        nc.gpsimd.memset(sep_pad[:, :, 0], 0.0)
        nc.gpsimd.memset(sep_pad[:, :, W + 1], 0.0)

        h_chunks = [4, 4, 8] + [16] * 7  # small first chunks: conv0 starts sooner
        h0 = 0
        for HC in h_chunks:
            # A-term: colRT[i,h] bcast over w;  B-term: c2[i,w] bcast over h
            cR = colRT[:, h0:h0 + HC]
            aR = bass.AP(tensor=cR.tensor, offset=cR.offset,
                         ap=[cR.ap[0], [1, HC], [0, W]])
            cJ = colJT[:, h0:h0 + HC]
            aJ = bass.AP(tensor=cJ.tensor, offset=cJ.offset,
                         ap=[cJ.ap[0], [1, HC], [0, W]])
            c2a = c2ated[:, :]
            b2 = bass.AP(tensor=c2a.tensor, offset=c2a.offset,
                         ap=[c2a.ap[0], [0, HC], [1, W]])
            s2a = s2ated[:, :]
            b3 = bass.AP(tensor=s2a.tensor, offset=s2a.offset,
                         ap=[s2a.ap[0], [0, HC], [1, W]])
            t1 = tsp.tile([128, 16, W], F16, tag="tsp", name="t1")[:, :HC, :]
            nc.gpsimd.tensor_tensor(out=t1[:], in0=aR, in1=b2, op=mybir.AluOpType.mult)
            t2 = tsp.tile([128, 16, W], F16, tag="tsp", name="t2")[:, :HC, :]
            nc.vector.tensor_tensor(out=t2[:], in0=aJ, in1=b3, op=mybir.AluOpType.mult)
            nc.vector.tensor_tensor(out=sep_pad[:, 1 + h0:1 + h0 + HC, 1:1 + W],
                                    in0=t1[:], in1=t2[:], op=mybir.AluOpType.add)
            h0 += HC

        # ---------------- conv0: 134 -> 256, 128x128 -> 64x64 ----------------
        conv0_pad = [act.tile([128, S0 + 2, S0 + 2], F16, tag=f"c0p{m}", name=f"c0p{m}") for m in range(2)]
        with low_priority(tc, 400):
            for m in range(2):
                nc.gpsimd.memset(conv0_pad[m][:, 0, :], 0.0)
                nc.gpsimd.memset(conv0_pad[m][:, S0 + 1, :], 0.0)
                nc.gpsimd.memset(conv0_pad[m][:, :, 0], 0.0)
                nc.gpsimd.memset(conv0_pad[m][:, :, S0 + 1], 0.0)

        OHB0 = 8  # oh rows per chunk -> N = 8*64 = 512
        for m in range(2):
            for ch in range(S0 // OHB0):
                oh0 = ch * OHB0
                p0 = ps.tile([128, OHB0, S0], F32, tag="ps")
                for t in range(16):
                    kh, kw = t // 4, t % 4
                    rhs = sep_pad[:, kh + 2 * oh0: kh + 2 * oh0 + 2 * OHB0 - 1: 2,
                                  kw: kw + 2 * S0 - 1: 2]
                    nc.tensor.matmul(p0[:], w0s_sb[:, m, t, :], rhs,
                                     start=(t == 0), stop=False)
                nc.tensor.matmul(p0[:], wm_sb[:, m * 128:(m + 1) * 128],
                                 mask_im[:, oh0:oh0 + OHB0, :],
                                 start=False, stop=True)
                # evac: lrelu(x + b0) = 0.2*(x+b0) + 0.8*relu(x+b0)
                relu_t = tmp.tile([128, OHB0, S0], F16, tag="ev")
                nc.scalar.activation(out=relu_t[:], in_=p0[:],
                                     func=mybir.ActivationFunctionType.Relu,
                                     bias=b0t[:, 2 + m:3 + m], scale=0.8)
                lin_t = tmp.tile([128, OHB0, S0], F16, tag="ev")
                nc.vector.tensor_scalar(out=lin_t[:], in0=p0[:],
                                        scalar1=b0t[:, m:m + 1], scalar2=0.2,
                                        op0=mybir.AluOpType.add,
                                        op1=mybir.AluOpType.mult)
                nc.vector.tensor_tensor(
                    out=conv0_pad[m][:, 1 + oh0:1 + oh0 + OHB0, 1:1 + S0],
                    in0=lin_t[:], in1=relu_t[:], op=mybir.AluOpType.add)

        # ---------------- generic strided conv layer with inorm ----------------
        def conv_norm(x_pads, w_dram, nm, nk, osz, out_pads, pooled=None,
                      wk_provider=None):
            """x_pads: list of nk input padded tiles (128, isz+2, isz+2) fp16
            w_dram: DRAM (nm, nk, 128, 16, 128) fp16
            out_pads: list of nm output padded tiles, or None with pooled tile."""
            n_spatial = osz * osz
            # chunk rows so N <= 512
            ohb = max(1, min(osz, 512 // osz))
            nch = osz // ohb
            for m in range(nm):
                pm = ps.tile([128, osz, osz], F32, tag="ps")
                if wk_provider is not None:
                    wk = [wk_provider(m, k) for k in range(nk)]
                else:
                    wk = []
                    for k in range(nk):
                        wt = wch.tile([128, 16, 128], F16, tag="wch")
                        nc.sync.dma_start(wt[:], w_dram.ap()[m, k])
                        wk.append(wt)
                for ch in range(nch):
                    oh0 = ch * ohb
                    pslice = pm[:, oh0:oh0 + ohb, :]
                    first = True
                    for k in range(nk):
                        for t in range(16):
                            kh, kw = t // 4, t % 4
                            rhs = x_pads[k][:, kh + 2 * oh0: kh + 2 * oh0 + 2 * ohb - 1: 2,
                                            kw: kw + 2 * osz - 1: 2]
                            nc.tensor.matmul(pslice, wk[k][:, t, :], rhs,
                                             start=first,
                                             stop=(k == nk - 1 and t == 15))
                            first = False
                # instance norm stats over full spatial
                nsub = max(1, n_spatial // 512)
                sub = n_spatial // nsub
                stats = tmp.tile([128, nsub, 6], F32, tag="st")
                pf = pm[:].rearrange("p a b -> p (a b)")
                for s in range(nsub):
                    nc.vector.bn_stats(out=stats[:, s, :], in_=pf[:, s * sub:(s + 1) * sub])
                mv = tmp.tile([128, 2], F32, tag="mv")
                nc.vector.bn_aggr(out=mv[:], in_=stats[:])
                eps_t = tmp.tile([128, 1], F32, tag="eps")
                nc.vector.memset(eps_t[:], EPS)
                rs = tmp.tile([128, 1], F32, tag="rs")
                nc.scalar.activation(out=rs[:], in_=mv[:, 1:2],
                                     func=mybir.ActivationFunctionType.Sqrt,
                                     bias=eps_t[:], scale=1.0)
                nc.vector.reciprocal(out=rs[:], in_=rs[:])
                rs08 = tmp.tile([128, 1], F32, tag="rs08")
                nc.vector.tensor_scalar_mul(out=rs08[:], in0=rs[:], scalar1=0.8)
                rs02 = tmp.tile([128, 1], F32, tag="rs02")
                nc.vector.tensor_scalar_mul(out=rs02[:], in0=rs[:], scalar1=0.2)
                nmrs = tmp.tile([128, 1], F32, tag="nmrs")
                nc.vector.tensor_tensor(out=nmrs[:], in0=mv[:, 0:1], in1=rs08[:],
                                        op=mybir.AluOpType.mult)
                nc.vector.tensor_scalar_mul(out=nmrs[:], in0=nmrs[:], scalar1=-1.0)

                if out_pads is not None:
                    relu_t = tmp.tile([128, osz, osz], F16, tag="ev")
                    nc.scalar.activation(out=relu_t[:], in_=pm[:],
                                         func=mybir.ActivationFunctionType.Relu,
                                         bias=nmrs[:], scale=rs08[:])
                    lin_t = tmp.tile([128, osz, osz], F16, tag="ev")
                    nc.vector.tensor_scalar(out=lin_t[:], in0=pm[:],
                                            scalar1=mv[:, 0:1], scalar2=rs02[:],
                                            op0=mybir.AluOpType.subtract,
                                            op1=mybir.AluOpType.mult)
                    nc.vector.tensor_tensor(
                        out=out_pads[m][:, 1:1 + osz, 1:1 + osz],
                        in0=lin_t[:], in1=relu_t[:], op=mybir.AluOpType.add)
                else:
                    # pooled output only: materialize normalized lrelu then reduce
                    relu_t = tmp.tile([128, osz * osz], F32, tag="ev3")
                    nc.scalar.activation(out=relu_t[:], in_=pf,
                                         func=mybir.ActivationFunctionType.Relu,
                                         bias=nmrs[:], scale=rs08[:])
                    lin_t = tmp.tile([128, osz * osz], F32, tag="ev3")
                    nc.vector.tensor_scalar(out=lin_t[:], in0=pf,
                                            scalar1=mv[:, 0:1], scalar2=rs02[:],
                                            op0=mybir.AluOpType.subtract,
                                            op1=mybir.AluOpType.mult)
                    both = tmp.tile([128, osz * osz], F32, tag="ev3")
                    nc.vector.tensor_tensor(out=both[:], in0=lin_t[:], in1=relu_t[:],
                                            op=mybir.AluOpType.add)
                    nc.vector.tensor_reduce(out=pooled[:, m:m + 1], in_=both[:],
                                            axis=mybir.AxisListType.X,
                                            op=mybir.AluOpType.add)

        # conv1: 256 -> 512, 64x64 -> 32x32
        conv1_pad = [act.tile([128, S1 + 2, S1 + 2], F16, tag=f"c1p{m}", name=f"c1p{m}") for m in range(4)]
        with low_priority(tc, 800):
            for m in range(4):
                nc.gpsimd.memset(conv1_pad[m][:, 0, :], 0.0)
                nc.gpsimd.memset(conv1_pad[m][:, S1 + 1, :], 0.0)
                nc.gpsimd.memset(conv1_pad[m][:, :, 0], 0.0)
                nc.gpsimd.memset(conv1_pad[m][:, :, S1 + 1], 0.0)
        conv_norm(conv0_pad, d_w1, 4, 2, S1, conv1_pad)

        # conv2: 512 -> 1024, 32x32 -> 16x16
        conv2_pad = [act.tile([128, S2 + 2, S2 + 2], F16, tag=f"c2p{m}", name=f"c2p{m}") for m in range(8)]
        with low_priority(tc, 1200):
            for m in range(8):
                nc.gpsimd.memset(conv2_pad[m][:, 0, :], 0.0)
                nc.gpsimd.memset(conv2_pad[m][:, S2 + 1, :], 0.0)
                nc.gpsimd.memset(conv2_pad[m][:, :, 0], 0.0)
                nc.gpsimd.memset(conv2_pad[m][:, :, S2 + 1], 0.0)
        conv_norm(conv1_pad, d_w2, 8, 4, S2, conv2_pad)

        # conv3: 1024 -> 1024, 16x16 -> 8x8; only pooled means survive
        # Preload w3 chunks into recycled slots (sep_pad / conv0_pad / mask_im /
        # tsp are dead by now) plus the wch pool, to keep DMA busy mid-kernel.
        c3w = {}
        idx = 0
        specs = [(act, "sep_pad", 8), (act, "c0p0", 2), (act, "c0p1", 2),
                 (act, "mask_im", 2), (tsp, "tsp", 1), (tsp, "tsp", 1),
                 (tsp, "tsp", 1), (act, "c1p0", 1), (act, "c1p1", 1),
                 (act, "c1p2", 1), (act, "c1p3", 1)]
        for pool_, tag_, n_ in specs:
            t = pool_.tile([128, n_, 16, 128], F16, tag=tag_, name=f"w3pre{idx}")
            for j in range(n_):
                m_, k_ = divmod(idx, 8)
                nc.sync.dma_start(t[:, j], d_w3.ap()[m_, k_])
                c3w[(m_, k_)] = t[:, j]
                idx += 1
        for rest in range(idx, 64):
            m_, k_ = divmod(rest, 8)
            t = wch.tile([128, 16, 128], F16, tag="wch", name=f"w3c{rest}")
            nc.sync.dma_start(t[:], d_w3.ap()[m_, k_])
            c3w[(m_, k_)] = t[:]
        pooled32 = const.tile([128, 8], F32)
        conv_norm(conv2_pad, d_w3, 8, 8, S3, None, pooled=pooled32,
                  wk_provider=lambda m, k: c3w[(m, k)])

        # head: out = w4^T @ pooled (w4 pre-scaled by 1/64) + b4
        pooled16 = const.tile([128, 8], F16)
        nc.vector.tensor_copy(pooled16[:], pooled32[:])
        pH = ps.tile([128, 1], F32, tag="ps")
        for k in range(8):
            nc.tensor.matmul(pH[:], w4_sb[:, k, :], pooled16[:, k:k + 1],
                             start=(k == 0), stop=(k == 7))
        out_sb = const.tile([128, 1], F32)
        nc.vector.tensor_tensor(out=out_sb[:], in0=pH[:], in1=b4t[:],
                                op=mybir.AluOpType.add)
        nc.sync.dma_start(d_out.ap(), out_sb[:])

    nc.compile()
    return nc


_NC = None


def _get_nc():
    global _NC
    if _NC is None:
        _NC = _build_nc()
    return _NC


def _prep_shared(w0, b0, w1, w2, w3, w4, b4):
    f16 = np.float16
    idx = np.arange(W)
    ang = (2.0 * np.pi / W) * np.outer(idx, idx).astype(np.float32)
    twC = (np.cos(ang) / W).astype(np.float32)
    twS = (np.sin(ang) / W).astype(np.float32)
    twc2 = np.cos(ang).astype(f16)
    tws2 = np.sin(ang).astype(f16)

    # w0 sep part: [m, t, ci, co]
    w0f = np.asarray(w0, np.float32)
    w0s = np.empty((2, 16, 128, 128), f16)
    for m in range(2):
        for t in range(16):
            kh, kw = t // 4, t % 4
            w0s[m, t] = w0f[128 * m:128 * (m + 1), 0:128, kh, kw].T.astype(f16)
    # w0 mask part: rows (kh,kw,ci) matching host im2col order, cols (m,co)
    wm = np.zeros((96, 2, 128), f16)
    for kh in range(4):
        for kw in range(4):
            for m in range(2):
                wm[(kh * 4 + kw) * 6:(kh * 4 + kw) * 6 + 6, m, :] = \
                    w0f[128 * m:128 * (m + 1), 128:134, kh, kw].T.astype(f16)
    wm = wm.reshape(96, 256)

    def pack(wl, nm, nk):
        wlf = np.asarray(wl, np.float32)
        o = np.empty((nm, nk, 128, 16, 128), f16)
        for m in range(nm):
            for k in range(nk):
                for t in range(16):
                    kh, kw = t // 4, t % 4
                    o[m, k, :, t, :] = wlf[128 * m:128 * (m + 1),
                                           128 * k:128 * (k + 1), kh, kw].T.astype(f16)
        return o

    w1l = pack(w1, 4, 2)
    w2l = pack(w2, 8, 4)
    w3l = pack(w3, 8, 8)
    w4f = np.asarray(w4, np.float32)[:, :, 0, 0] / (S3 * S3)  # (128, 1024)
    w4l = np.empty((8, 128, 128), f16)
    for k in range(8):
        w4l[k] = w4f[:, 128 * k:128 * (k + 1)].T.astype(f16)

    b0f = np.asarray(b0, np.float32)
    b0t = np.stack([b0f[0:128], b0f[128:256], 0.8 * b0f[0:128], 0.8 * b0f[128:256]],
                   axis=1).astype(np.float32)  # (128, 4)
    b4t = np.asarray(b4, np.float32).reshape(128, 1)
    maskp_proto = None
    return dict(twC=twC, twS=twS, twc2=twc2, tws2=tws2, w0s=w0s, wm=wm,
                w1l=w1l, w2l=w2l, w3l=w3l, w4l=w4l, b0t=b0t, b4t=b4t)


def kernel(image, mask_embedding, w0, b0, w1, b1, w2, b2, w3, b3, w4, b4):
    from concourse.bass_utils import run_bass_kernel_spmd

    nc = _get_nc()
    shared = _prep_shared(w0, b0, w1, w2, w3, w4, b4)

    image = np.asarray(image, np.float32)
    mask = np.asarray(mask_embedding, np.float32)
    in_maps = []
    for b in range(B):
        mp = np.zeros((6, H + 2, W + 2), np.float16)
        mp[:, 1:H + 1, 1:W + 1] = mask[b].astype(np.float16)
        imcol = np.empty((96, S0, S0), np.float16)
        for kh in range(4):
            for kw in range(4):
                t = kh * 4 + kw
                imcol[t * 6:(t + 1) * 6] = mp[:, kh:kh + 2 * S0 - 1:2,
                                              kw:kw + 2 * S0 - 1:2]
        m = dict(shared)
        m["img"] = image[b, 0]
        m["maskim"] = imcol
        in_maps.append(m)

    res = run_bass_kernel_spmd(nc, in_maps, list(range(B)))
    out = np.stack([res.results[b]["out"] for b in range(B)]).astype(np.float32)
    return out



# revision 10
# speedup vs baseline: 1.6222x; 1.6222x over previous
"""Trainium2 Bass kernel for nn_EvaluatorNetwork (fp8 DoubleRow version).

Network (per sample):
  sep = per-column spectral decomposition of image  -> (128, 128, 128)
  x = concat([sep, mask_embedding]) -> (134, 128, 128)
  conv0 4x4 s2 (134->256) + b0, lrelu          -> (256, 64, 64)
  conv1 4x4 s2 (256->512), inorm, lrelu        -> (512, 32, 32)
  conv2 4x4 s2 (512->1024), inorm, lrelu       -> (1024, 16, 16)
  conv3 4x4 s2 (1024->1024), inorm, lrelu      -> (1024, 8, 8)
  avgpool -> (1024,); head 1024->128 + b4      -> (128,)

Sharding: pure data parallel, batch 8 over 8 NeuronCores; weights replicated.

fp8 strategy (float8e4 = e4m3, max finite 240):
  All convs run as fp8 DoubleRow matmuls (2 vertical taps (kh, kh+2) fused
  per instruction -> contract 256, 2x PE rate, half the weight DMA bytes).
  Scaling is free because conv+instancenorm is invariant to any per-output-
  channel scale:
    - weights quantized with per-out-channel scale s = 160/max|w| (cancels
      in inorm; for conv0, which has no inorm, 1/s is folded into the evac
      constants)
    - activations stored as 16*lrelu(...) (the 16 cancels in the next inorm)
    - sep stored as 64*sep (64 baked into the twiddle matrices; divided out
      in conv0's evac constants)

Parity-split activation layout (needed so the DoubleRow moving operand is
an exact 3D [K, 2, N] access pattern):
  A conv input of spatial size isz=2*on is stored as [128ch, 2pr, 2pc,
  2c01, on+1, on] fp8, indexed by padded coords ih, iw in [-1, isz]:
    pr = ih&1, rr = (ih+1)>>1;  pc = iw&1, jj = (iw+1)>>1
    copy c01=0 holds cols jj in [0, on); copy c01=1 holds jj in [1, on]
    at col jj-1 (column data is stored twice, shifted by one).
  Tap (kh, kw) of a stride-2 4x4 conv then reads the contiguous block
    [:, (kh+1)&1, (kw+1)&1, kw>>1, oh0+(kh>>1) : +ohb, 0:on]
  whose (rows, cols) merge into one N dim, and the vertical pair partner
  (kh+2, kw) is exactly one row (+on elements) below -> the DoubleRow pair
  dim is [stride=on, 2].
  b1..b3 cancel exactly through instance norm; b0 and b4 are applied.
  lrelu(y) = 0.2*y + 0.8*relu(y); relu(a*y) = a*relu(y) for a>0.
"""
from contextlib import ExitStack

import numpy as np

import concourse.bass as bass
import concourse.tile as tile
from concourse import bacc, mybir
from concourse.masks import make_identity

F32 = mybir.dt.float32
F16 = mybir.dt.float16
F8 = mybir.dt.float8e4

B, H, W = 8, 128, 128
EPS = 1e-5

# conv output spatial sizes
S0, S1, S2, S3 = 64, 32, 16, 8

DR = mybir.MatmulPerfMode.DoubleRow


def _build_nc():
    nc = bacc.Bacc("TRN2", target_bir_lowering=False, debug=False)

    # ---------------- DRAM parameters (per-core) ----------------
    d_img = nc.dram_tensor("img", [H, W], F32, kind="ExternalInput")
    d_maskim = nc.dram_tensor("maskim", [96, S0, S0], F16, kind="ExternalInput")
    d_C = nc.dram_tensor("twC", [W, W], F32, kind="ExternalInput")
    d_S = nc.dram_tensor("twS", [W, W], F32, kind="ExternalInput")
    d_c2 = nc.dram_tensor("twc2", [W, W], F16, kind="ExternalInput")
    d_s2 = nc.dram_tensor("tws2", [W, W], F16, kind="ExternalInput")
    d_w0s = nc.dram_tensor("w0s", [128, 2, 8, 2, 128], F8, kind="ExternalInput")
    d_wm = nc.dram_tensor("wm", [96, 256], F16, kind="ExternalInput")
    d_w1 = nc.dram_tensor("w1l", [128, 4, 2, 8, 2, 128], F8, kind="ExternalInput")
    d_w2 = nc.dram_tensor("w2l", [128, 8, 4, 8, 2, 128], F8, kind="ExternalInput")
    # w3 split into 16 half-m groups (m, k-half) for streamed 1MB DMAs
    d_w3 = nc.dram_tensor("w3l", [16, 128, 4, 8, 2, 128], F8, kind="ExternalInput")
    d_w4 = nc.dram_tensor("w4l", [8, 128, 128], F16, kind="ExternalInput")
    d_b0 = nc.dram_tensor("b0t", [128, 8], F32, kind="ExternalInput")
    d_b4 = nc.dram_tensor("b4t", [128, 1], F32, kind="ExternalInput")
    d_out = nc.dram_tensor("out", [128], F32, kind="ExternalOutput")

    from contextlib import contextmanager

    @contextmanager
    def low_priority(tc, bump):
        orig = tc.cur_priority
        tc.cur_priority = orig + bump
        try:
            yield
        finally:
            tc.cur_priority = orig

    with tile.TileContext(nc) as tc, ExitStack() as ctx:
        const = ctx.enter_context(tc.tile_pool(name="const", bufs=1))
        act = ctx.enter_context(tc.tile_pool(name="act", bufs=1))
        wc3 = ctx.enter_context(tc.tile_pool(name="wc3", bufs=4))
        ps = ctx.enter_context(tc.tile_pool(name="ps", bufs=3, space="PSUM"))
        tmp = ctx.enter_context(tc.tile_pool(name="tmp", bufs=6))
        tsp = ctx.enter_context(tc.tile_pool(name="tsp", bufs=3))

        # ---------------- constants / inputs ----------------
        ident = const.tile([128, 128], F32)
        make_identity(nc, ident[:])
        img32 = const.tile([128, 128], F32)
        nc.sync.dma_start(img32[:], d_img.ap())
        twC = const.tile([128, 128], F32)
        nc.sync.dma_start(twC[:], d_C.ap())
        twS = const.tile([128, 128], F32)
        nc.sync.dma_start(twS[:], d_S.ap())
        c2ated = const.tile([128, 128], F16)
        nc.sync.dma_start(c2ated[:], d_c2.ap())
        s2ated = const.tile([128, 128], F16)
        nc.sync.dma_start(s2ated[:], d_s2.ap())
        b0t = const.tile([128, 8], F32)
        nc.sync.dma_start(b0t[:], d_b0.ap())
        b4t = const.tile([128, 1], F32)
        nc.sync.dma_start(b4t[:], d_b4.ap())

        # mask im2col (host-computed): rows (kh,kw,ci), cols (oh,ow)
        mask_im = act.tile([96, S0, S0], F16, name="mask_im")
        nc.sync.dma_start(mask_im[:], d_maskim.ap())

        # small weights resident
        w0s_sb = const.tile([128, 2, 8, 2, 128], F8)
        nc.sync.dma_start(w0s_sb[:], d_w0s.ap())
        wm_sb = const.tile([96, 256], F16)
        nc.sync.dma_start(wm_sb[:], d_wm.ap())
        w4_sb = const.tile([128, 8, 128], F16)
        nc.sync.dma_start(w4_sb[:], d_w4.ap().rearrange("t k c -> k t c"))
        # w1 and w2 fully resident, one batched DMA each
        w1_sb = const.tile([128, 4, 2, 8, 2, 128], F8)
        nc.sync.dma_start(w1_sb[:], d_w1.ap())
        w2_sb = const.tile([128, 8, 4, 8, 2, 128], F8)
        nc.sync.dma_start(w2_sb[:], d_w2.ap())

        # ---------- parity-tile helpers ----------
        def parity_tile(pool, on, name, tag=None):
            kw = dict(tag=tag) if tag else {}
            return pool.tile([128, 2, 2, 2, on + 1, on], F8, name=name, **kw)

        def parity_borders(t, on):
            # pads: ih=-1 (pr1 row0), ih=isz (pr0 row on),
            #       iw=-1 (pc1 A col0), iw=isz (pc0 B col on-1)
            for pc in (0, 1):
                for c01 in (0, 1):
                    nc.gpsimd.memset(t[:, 1, pc, c01, 0, :], 0.0)
                    nc.gpsimd.memset(t[:, 0, pc, c01, on, :], 0.0)
            for pr in (0, 1):
                nc.gpsimd.memset(t[:, pr, 1, 0, :, 0], 0.0)
                nc.gpsimd.memset(t[:, pr, 0, 1, :, on - 1], 0.0)

        # col-slice table: (pc, c01) -> (src_start, ncol, dst_col)
        def col_slices(on):
            return {(0, 0): (0, on, 0), (0, 1): (2, on - 1, 0),
                    (1, 0): (1, on - 1, 1), (1, 1): (1, on, 0)}

        def parity_add(dst, on, lin_t, relu_t, osz):
            """dst parity tile gets lin_t+relu_t ([128, osz, osz], osz=2*on)."""
            cs = col_slices(on)
            for pr in (0, 1):
                rd0 = 0 if pr == 0 else 1
                for pc in (0, 1):
                    for c01 in (0, 1):
                        sc, ncol, dc = cs[(pc, c01)]
                        sl = (slice(None), slice(pr, osz, 2),
                              slice(sc, sc + 2 * ncol - 1, 2))
                        nc.vector.tensor_tensor(
                            out=dst[:, pr, pc, c01, rd0:rd0 + on, dc:dc + ncol],
                            in0=lin_t[sl], in1=relu_t[sl],
                            op=mybir.AluOpType.add)

        def tap_rhs(xt, on, kh, kw, oh0, ohb):
            """Moving AP for vertical tap pair (kh,kw)&(kh+2,kw), kh in {0,1}:
            [128, 2(pair, stride on), ohb rows, on cols]."""
            sl = xt[:, (kh + 1) & 1, (kw + 1) & 1, kw >> 1,
                    oh0 + (kh >> 1): oh0 + (kh >> 1) + ohb, :]
            return bass.AP(tensor=sl.tensor, offset=sl.offset,
                           ap=[sl.ap[0], [on, 2], sl.ap[1], sl.ap[2]])

        # ---------------- spectral map ----------------
        pT = ps.tile([128, 128], F32, tag="ps")
        nc.tensor.transpose(pT[:], img32[:], ident[:])
        imgT32 = const.tile([128, 128], F32)
        nc.vector.tensor_copy(imgT32[:], pT[:])

        # colRT/colJT are 64x the true column transforms (64 baked into twC/twS)
        pR = ps.tile([128, 128], F32, tag="ps")
        nc.tensor.matmul(pR[:], twC[:], imgT32[:], start=True, stop=True)
        colRT = const.tile([128, 128], F16)
        nc.vector.tensor_copy(colRT[:], pR[:])
        pJ = ps.tile([128, 128], F32, tag="ps")
        nc.tensor.matmul(pJ[:], twS[:], imgT32[:], start=True, stop=True)
        colJT = const.tile([128, 128], F16)
        nc.vector.tensor_copy(colJT[:], pJ[:])

        # sep parity tile: input of conv0, on=64
        sep_t = parity_tile(act, S0, "sep_t")
        with low_priority(tc, 200):
            parity_borders(sep_t, S0)

        cs64 = col_slices(S0)
        h_chunks = [4, 4, 8] + [16] * 7  # small first chunks: conv0 starts sooner
        h0 = 0
        for HC in h_chunks:
            # A-term: colRT[i,h] bcast over w;  B-term: c2[i,w] bcast over h
            cR = colRT[:, h0:h0 + HC]
            aR = bass.AP(tensor=cR.tensor, offset=cR.offset,
                         ap=[cR.ap[0], [1, HC], [0, W]])
            cJ = colJT[:, h0:h0 + HC]
            aJ = bass.AP(tensor=cJ.tensor, offset=cJ.offset,
                         ap=[cJ.ap[0], [1, HC], [0, W]])
            c2a = c2ated[:, :]
            b2 = bass.AP(tensor=c2a.tensor, offset=c2a.offset,
                         ap=[c2a.ap[0], [0, HC], [1, W]])
            s2a = s2ated[:, :]
            b3 = bass.AP(tensor=s2a.tensor, offset=s2a.offset,
                         ap=[s2a.ap[0], [0, HC], [1, W]])
            t1 = tsp.tile([128, 16, W], F16, tag="tsp", name="t1")[:, :HC, :]
            nc.gpsimd.tensor_tensor(out=t1[:], in0=aR, in1=b2, op=mybir.AluOpType.mult)
            t2 = tsp.tile([128, 16, W], F16, tag="tsp", name="t2")[:, :HC, :]
            nc.vector.tensor_tensor(out=t2[:], in0=aJ, in1=b3, op=mybir.AluOpType.mult)
            # scatter the sum into the sep parity tile (8 strided adds)
            for pr in (0, 1):
                nr = HC // 2
                rd0 = h0 // 2 + (0 if pr == 0 else 1)
                for pc in (0, 1):
                    for c01 in (0, 1):
                        sc, ncol, dc = cs64[(pc, c01)]
                        sl = (slice(None), slice(pr, HC, 2),
                              slice(sc, sc + 2 * ncol - 1, 2))
                        nc.vector.tensor_tensor(
                            out=sep_t[:, pr, pc, c01, rd0:rd0 + nr, dc:dc + ncol],
                            in0=t1[sl], in1=t2[sl], op=mybir.AluOpType.add)
            h0 += HC

        # ---------------- conv0: 134 -> 256, 128x128 -> 64x64 ----------------
        # psum = 64*s0*conv0; evac produces 16*lrelu(conv0+b0) in fp8 parity
        # tiles that are conv1's two input-channel groups.
        c1in = [parity_tile(act, S1, f"c1in{m}") for m in range(2)]
        with low_priority(tc, 400):
            for m in range(2):
                parity_borders(c1in[m], S1)

        OHB0 = 8  # oh rows per chunk -> N = 8*64 = 512
        for m in range(2):
            for ch in range(S0 // OHB0):
                oh0 = ch * OHB0
                p0 = ps.tile([128, OHB0, S0], F32, tag="ps")
                nc.tensor.matmul(p0[:], wm_sb[:, m * 128:(m + 1) * 128],
                                 mask_im[:, oh0:oh0 + OHB0, :],
                                 start=True, stop=False)
                for p in range(8):
                    kh, kw = p >> 2, p & 3
                    nc.tensor.matmul(p0[:], w0s_sb[:, m, p],
                                     tap_rhs(sep_t, S0, kh, kw, oh0, OHB0),
                                     start=False, stop=(p == 7), perf_mode=DR)
                # evac: 16*lrelu(y+b0) = 3.2*(y+b0) + 12.8*relu(y+b0),
                # y = p0/(64*s0)
                relu_t = tmp.tile([128, OHB0, S0], F8, tag="ev")
                nc.scalar.activation(out=relu_t[:], in_=p0[:],
                                     func=mybir.ActivationFunctionType.Relu,
                                     bias=b0t[:, 2 + m:3 + m],
                                     scale=b0t[:, 6 + m:7 + m])
                lin_t = tmp.tile([128, OHB0, S0], F8, tag="ev")
                nc.vector.tensor_scalar(out=lin_t[:], in0=p0[:],
                                        scalar1=b0t[:, m:m + 1],
                                        scalar2=b0t[:, 4 + m:5 + m],
                                        op0=mybir.AluOpType.add,
                                        op1=mybir.AluOpType.mult)
                # chunk rows oh0..oh0+7 scatter into c1in[m] (on=32)
                cs32 = col_slices(S1)
                for pr in (0, 1):
                    nr = OHB0 // 2
                    rd0 = oh0 // 2 + (0 if pr == 0 else 1)
                    for pc in (0, 1):
                        for c01 in (0, 1):
                            sc, ncol, dc = cs32[(pc, c01)]
                            sl = (slice(None), slice(pr, OHB0, 2),
                                  slice(sc, sc + 2 * ncol - 1, 2))
                            nc.vector.tensor_tensor(
                                out=c1in[m][:, pr, pc, c01,
                                            rd0:rd0 + nr, dc:dc + ncol],
                                in0=lin_t[sl], in1=relu_t[sl],
                                op=mybir.AluOpType.add)

        # ---------------- generic strided conv layer with inorm ----------------
        def conv_norm(x_tiles, w_dram, nm, nk, osz, out_tiles, pooled=None,
                      wk_provider=None):
            """x_tiles: nk input parity tiles (on_in = osz); out m-tile -> the
            next layer's k-tile m parity tile (on = osz//2), or pooled."""
            n_spatial = osz * osz
            on_in = osz
            ohb = max(1, min(osz, 512 // osz))
            nch = osz // ohb
            for m in range(nm):
                pm = ps.tile([128, osz, osz], F32, tag="ps")
                if wk_provider is not None:
                    wk = [wk_provider(m, k) for k in range(nk)]
                else:
                    wk = []
                    for k in range(nk):
                        wt = wch.tile([128, 8, 2, 128], F8, tag="wch")
                        nc.sync.dma_start(wt[:], w_dram.ap()[m, k])
                        wk.append(wt)
                for ch in range(nch):
                    oh0 = ch * ohb
                    pslice = pm[:, oh0:oh0 + ohb, :]
                    first = True
                    for k in range(nk):
                        for p in range(8):
                            kh, kw = p >> 2, p & 3
                            nc.tensor.matmul(
                                pslice, wk[k][:, p],
                                tap_rhs(x_tiles[k], on_in, kh, kw, oh0, ohb),
                                start=first,
                                stop=(k == nk - 1 and p == 7),
                                perf_mode=DR)
                            first = False
                # instance norm stats over full spatial (scale-invariant)
                nsub = max(1, n_spatial // 512)
                sub = n_spatial // nsub
                stats = tmp.tile([128, nsub, 6], F32, tag="st")
                pf = pm[:].rearrange("p a b -> p (a b)")
                for s in range(nsub):
                    nc.vector.bn_stats(out=stats[:, s, :], in_=pf[:, s * sub:(s + 1) * sub])
                mv = tmp.tile([128, 2], F32, tag="mv")
                nc.vector.bn_aggr(out=mv[:], in_=stats[:])
                eps_t = tmp.tile([128, 1], F32, tag="eps")
                nc.vector.memset(eps_t[:], EPS)
                rs = tmp.tile([128, 1], F32, tag="rs")
                nc.scalar.activation(out=rs[:], in_=mv[:, 1:2],
                                     func=mybir.ActivationFunctionType.Sqrt,
                                     bias=eps_t[:], scale=1.0)
                nc.vector.reciprocal(out=rs[:], in_=rs[:])
                if out_tiles is not None:
                    ra, rb = 12.8, 3.2  # out = 16*lrelu(inorm(x))
                else:
                    ra, rb = 0.8, 0.2  # out = lrelu(inorm(x))
                rsA = tmp.tile([128, 1], F32, tag="rs08")
                nc.vector.tensor_scalar_mul(out=rsA[:], in0=rs[:], scalar1=ra)
                rsB = tmp.tile([128, 1], F32, tag="rs02")
                nc.vector.tensor_scalar_mul(out=rsB[:], in0=rs[:], scalar1=rb)
                nmrs = tmp.tile([128, 1], F32, tag="nmrs")
                nc.vector.tensor_tensor(out=nmrs[:], in0=mv[:, 0:1], in1=rsA[:],
                                        op=mybir.AluOpType.mult)
                nc.vector.tensor_scalar_mul(out=nmrs[:], in0=nmrs[:], scalar1=-1.0)

                if out_tiles is not None:
                    relu_t = tmp.tile([128, osz, osz], F8, tag="ev")
                    nc.scalar.activation(out=relu_t[:], in_=pm[:],
                                         func=mybir.ActivationFunctionType.Relu,
                                         bias=nmrs[:], scale=rsA[:])
                    lin_t = tmp.tile([128, osz, osz], F8, tag="ev")
                    nc.vector.tensor_scalar(out=lin_t[:], in0=pm[:],
                                            scalar1=mv[:, 0:1], scalar2=rsB[:],
                                            op0=mybir.AluOpType.subtract,
                                            op1=mybir.AluOpType.mult)
                    parity_add(out_tiles[m], osz // 2, lin_t, relu_t, osz)
                else:
                    # pooled output only: materialize normalized lrelu then reduce
                    relu_t = tmp.tile([128, osz * osz], F32, tag="ev3")
                    nc.scalar.activation(out=relu_t[:], in_=pf,
                                         func=mybir.ActivationFunctionType.Relu,
                                         bias=nmrs[:], scale=rsA[:])
                    lin_t = tmp.tile([128, osz * osz], F32, tag="ev3")
                    nc.vector.tensor_scalar(out=lin_t[:], in0=pf,
                                            scalar1=mv[:, 0:1], scalar2=rsB[:],
                                            op0=mybir.AluOpType.subtract,
                                            op1=mybir.AluOpType.mult)
                    both = tmp.tile([128, osz * osz], F32, tag="ev3")
                    nc.vector.tensor_tensor(out=both[:], in0=lin_t[:], in1=relu_t[:],
                                            op=mybir.AluOpType.add)
                    nc.vector.tensor_reduce(out=pooled[:, m:m + 1], in_=both[:],
                                            axis=mybir.AxisListType.X,
                                            op=mybir.AluOpType.add)

        # conv1: 256 -> 512, 64x64 -> 32x32 (outputs are conv2's 4 k-tiles)
        c2in = [parity_tile(act, S2, f"c2in{m}") for m in range(4)]
        with low_priority(tc, 800):
            for m in range(4):
                parity_borders(c2in[m], S2)
        conv_norm(c1in, d_w1, 4, 2, S1, c2in)

        # conv2: 512 -> 1024, 32x32 -> 16x16 (outputs are conv3's 8 k-tiles)
        c3in = [parity_tile(act, S3, f"c3in{m}") for m in range(8)]
        with low_priority(tc, 1200):
            for m in range(8):
                parity_borders(c3in[m], S3)
        conv_norm(c2in, d_w2, 8, 4, S2, c3in)

        # conv3: 1024 -> 1024, 16x16 -> 8x8; only pooled means survive
        # Preload w3 (m,k)-tiles into recycled slots (sep_t / c1in / mask_im /
        # tsp are dead by then) plus the wch pool, to keep DMA busy mid-kernel.
        # 64 tiles of (128, 8, 2, 128) fp8 = 2KB/partition each.
        c3w = {}
        idx = 0
        specs = [(act, "sep_t", 16), (act, "c1in0", 4), (act, "c1in1", 4),
                 (act, "mask_im", 4), (tsp, "tsp", 2), (tsp, "tsp", 2),
                 (tsp, "tsp", 2)]
        for pool_, tag_, n_ in specs:
            t = pool_.tile([128, n_, 8, 2, 128], F8, tag=tag_, name=f"w3pre{idx}")
            for j in range(n_):
                m_, k_ = divmod(idx, 8)
                nc.sync.dma_start(t[:, j], d_w3.ap()[m_, k_])
                c3w[(m_, k_)] = t[:, j]
                idx += 1
        for rest in range(idx, 64):
            m_, k_ = divmod(rest, 8)
            t = wch.tile([128, 8, 2, 128], F8, tag="wch", name=f"w3c{rest}")
            nc.sync.dma_start(t[:], d_w3.ap()[m_, k_])
            c3w[(m_, k_)] = t[:]
        pooled32 = const.tile([128, 8], F32)
        conv_norm(c3in, d_w3, 8, 8, S3, None, pooled=pooled32,
                  wk_provider=lambda m, k: c3w[(m, k)])

        # head: out = w4^T @ pooled (w4 pre-scaled by 1/64) + b4
        pooled16 = const.tile([128, 8], F16)
        nc.vector.tensor_copy(pooled16[:], pooled32[:])
        pH = ps.tile([128, 1], F32, tag="ps")
        for k in range(8):
            nc.tensor.matmul(pH[:], w4_sb[:, k, :], pooled16[:, k:k + 1],
                             start=(k == 0), stop=(k == 7))
        out_sb = const.tile([128, 1], F32)
        nc.vector.tensor_tensor(out=out_sb[:], in0=pH[:], in1=b4t[:],
                                op=mybir.AluOpType.add)
        nc.sync.dma_start(d_out.ap(), out_sb[:])

    nc.compile()
    return nc


_NC = None


def _get_nc():
    global _NC
    if _NC is None:
        _NC = _build_nc()
    return _NC


def _prep_shared(w0, b0, w1, w2, w3, w4, b4):
    f16 = np.float16
    f8 = mybir.dt.np(F8)
    W_TGT = 160.0  # per-out-channel weight scale target (e4m3 max finite 240)

    idx = np.arange(W)
    ang = (2.0 * np.pi / W) * np.outer(idx, idx).astype(np.float32)
    twC = (64.0 * np.cos(ang) / W).astype(np.float32)
    twS = (64.0 * np.sin(ang) / W).astype(np.float32)
    twc2 = np.cos(ang).astype(f16)
    tws2 = np.sin(ang).astype(f16)

    # conv0 sep part: lhsT[i, m, p, j, c] = s0*w0[m*128+c, i, kh+2j, kw],
    # pair p = kh*4+kw with kh in {0,1}
    w0f = np.asarray(w0, np.float32)
    s0 = W_TGT / np.abs(w0f[:, :128]).max(axis=(1, 2, 3))  # (256,)
    ws0 = (w0f[:, :128] * s0[:, None, None, None]).reshape(2, 128, 128, 4, 4)
    # [m, c, i, KH, KW] -> KH=(j,kh) -> [i, m, kh, kw, j, c]
    w5 = ws0.reshape(2, 128, 128, 2, 2, 4)  # [m, c, i, j, kh, kw]
    w0s = np.ascontiguousarray(w5.transpose(2, 0, 4, 5, 3, 1))  # [i,m,kh,kw,j,c]
    w0s = w0s.reshape(128, 2, 8, 2, 128).astype(f8)

    # w0 mask part: rows (kh,kw,ci) matching host im2col order, cols (m,co);
    # carries 64*s0 so it lands on the same psum scale as the sep part
    wmf = w0f[:, 128:134]  # (256, 6, 4, 4)
    wm = (wmf.transpose(2, 3, 1, 0) * (64.0 * s0)[None, None, None, :]) \
        .reshape(96, 256).astype(f16)

    def pack(wl, nm, nk):
        wlf = np.asarray(wl, np.float32)
        s = W_TGT / np.abs(wlf).max(axis=(1, 2, 3))
        wsc = wlf * s[:, None, None, None]
        # o[m, k, kk, p=(kh,kw), j, c] = wsc[m*128+c, k*128+kk, kh+2j, kw]
        w6 = wsc.reshape(nm, 128, nk, 128, 2, 2, 4)  # [m, c, k, kk, j, kh, kw]
        o = np.ascontiguousarray(w6.transpose(0, 2, 3, 5, 6, 4, 1))
        # [m, k, kk, kh, kw, j, c]
        return o.reshape(nm, nk, 128, 8, 2, 128).astype(f8)

    w1l = pack(w1, 4, 2)
    w2l = pack(w2, 8, 4)
    w3l = pack(w3, 8, 8)
    w4f = np.asarray(w4, np.float32)[:, :, 0, 0] / (S3 * S3)  # (128, 1024)
    w4l = np.empty((8, 128, 128), f16)
    for k in range(8):
        w4l[k] = w4f[:, 128 * k:128 * (k + 1)].T.astype(f16)

    # b0t cols: [64*s0*b0 (m0,m1), 12.8*b0 (m0,m1), 0.05/s0 (m0,m1), 0.2/s0 (m0,m1)]
    b0f = np.asarray(b0, np.float32)
    b0m = b0f.reshape(2, 128).T  # (128, 2)
    s0m = s0.reshape(2, 128).T  # (128, 2)
    b0t = np.concatenate([64.0 * s0m * b0m, 12.8 * b0m,
                          0.05 / s0m, 0.2 / s0m], axis=1).astype(np.float32)
    b4t = np.asarray(b4, np.float32).reshape(128, 1)
    return dict(twC=twC, twS=twS, twc2=twc2, tws2=tws2, w0s=w0s, wm=wm,
                w1l=w1l, w2l=w2l, w3l=w3l, w4l=w4l, b0t=b0t, b4t=b4t)


def kernel(image, mask_embedding, w0, b0, w1, b1, w2, b2, w3, b3, w4, b4):
    from concourse.bass_utils import run_bass_kernel_spmd

    nc = _get_nc()
    shared = _prep_shared(w0, b0, w1, w2, w3, w4, b4)

    image = np.asarray(image, np.float32)
    mask = np.asarray(mask_embedding, np.float32)
    in_maps = []
    for b in range(B):
        mp = np.zeros((6, H + 2, W + 2), np.float16)
        mp[:, 1:H + 1, 1:W + 1] = mask[b].astype(np.float16)
        imcol = np.empty((96, S0, S0), np.float16)
        for kh in range(4):
            for kw in range(4):
                t = kh * 4 + kw
                imcol[t * 6:(t + 1) * 6] = mp[:, kh:kh + 2 * S0 - 1:2,
                                              kw:kw + 2 * S0 - 1:2]
        m = dict(shared)
        m["img"] = image[b, 0]
        m["maskim"] = imcol
        in_maps.append(m)

    res = run_bass_kernel_spmd(nc, in_maps, list(range(B)))
    out = np.stack([res.results[b]["out"] for b in range(B)]).astype(np.float32)
    return out
